# revision 1
# baseline (speedup 1.0000x reference)
"""Trainium2 Bass kernel for nn_BaselineModel_27298812133937.

Model: two [32,512] token sequences -> shared embedding [50000,512] ->
3 stacked bi-GRU layers (H=256, Keras reset_after) -> last states,
plus a leaks MLP branch, then BN/FC/BN/sigmoid head -> [32].

Sharding: the two sequences share GRU weights, so they merge into a
batch of 64. Each of the 8 cores takes 8 merged examples (4 code + 4
comment of the same original examples), runs the full network for its
shard with no cross-core communication, and computes the head for its
4 original examples. Host concatenates the 8x[4] outputs.

On-core layout: feature dim on partitions, batch on the free dim. The
recurrent matmul keeps Wh (bf16) stationary and streams the state.

Design (measured 4.16 ms at full 2.4 GHz clock vs 7.48 ms baseline;
rel err 8.2e-4):
- All layer activations (x) and input projections (xp) are SBUF-resident
  (bf16): the projection writes PSUM->SBUF directly and the scan reads
  xp slices directly, eliminating ~40 MB/core of DRAM round-trips that
  throttled the single SP DMA queue (~18 GB/s effective).
- Scan step: xz (+folded input/recurrent z,r biases) and the recurrent
  h-bias are accumulated into PSUM by identity-stationary matmuls.
  Per-dir PSUM is split into an r-tile (computed FIRST: sigmoid_r ->
  hm starts ~600ns before the burst ends) and a z+h tile; z-gates are
  off the critical path. Update form h' = z*h + (1-z)*hh (1-z via DVE)
  keeps the post-tanh path at 2 hops and improves accuracy 5x vs
  h' = hh + z*(h-hh) (no cancellation).
- Hidden state lives in a [P, KH, U, BC] SBUF ring (bw written
  time-reversed) copied on-chip into x once per half-block.
- Fully unrolled (no For_i): removes per-iteration ACT_TABLE_LOAD
  (1.3 us) and COMPARE_BRANCH overhead.
- Projection runs kt-outer over 4-chunk groups so each Wx stationary
  load serves 4 N=512 matmuls (4x fewer proj LDWEIGHTS).
The remaining wall is the PE weight-load path: 24+4 LDWEIGHTS per step
at ~104 ns each (no FWL control from Bass, and col-tiling the loads
crashes on the quadrant-3 HW bug), plus residual gate-chain stalls the
fw/bw stagger cannot fully hide. Next lever if continued: split
directions across core pairs (12 distinct loads/step/core) with
RDMA exchange of layer outputs.
"""

import os
import sys

import numpy as np

for _p in ("/opt/trn_rl_repo",):
    if os.path.isdir(_p) and _p not in sys.path:
        sys.path.insert(0, _p)

import concourse.bass as bass
import concourse.tile as tile
from concourse import bacc, mybir
from concourse.masks import make_identity

import ml_dtypes

FP32 = mybir.dt.float32
BF16 = mybir.dt.bfloat16
I32 = mybir.dt.int32
AF = mybir.ActivationFunctionType
OP = mybir.AluOpType
NP_BF16 = ml_dtypes.bfloat16

V, E, H, NLAY = 50000, 512, 256, 3
EPS = 1e-3
P = 128
JX = E // P        # 4  x-feature tiles
JG = 3 * H // P    # 6  gate tiles
JH = H // P        # 2  hidden tiles
KH = H // P        # 2  Wh contraction tiles
BC = 8             # merged examples per core
BCH = 4            # head (original) examples per core
NCORES = 8
U = 16             # scan steps per For_i iteration == xp time-block


def build_nc(T=512, n_layers=NLAY, use_for_i=True, staggered=True, debug=False):
    assert T % P == 0 and T % U == 0
    TB = T // U
    NCH = BC * (T // P)

    nc = bacc.Bacc("TRN2", target_bir_lowering=False, debug=debug)

    def din(name, shape, dt):
        return nc.declare_dram_parameter(name, list(shape), dt, False)

    emb = din("emb", [V, E], BF16)
    idxw = din("idxw", [P, NCH], I32)
    wx = din("wx", [n_layers, 2, JX, JG, P, P], BF16)
    wh = din("wh", [n_layers, 2, KH, JG, P, P], BF16)
    pbias = din("pbias", [P, n_layers, 2, JG], FP32)
    b1hbc = din("b1hbc", [P, n_layers, 2, JH, BC], BF16)
    w1 = din("w1", [10, 2, P, P], BF16)
    b1p = din("b1p", [P, 2], FP32)
    wc = din("wc", [P, 2], BF16)
    bc_b = din("bc", [1, 1], FP32)
    lw0 = din("lw0", [P, 2, P], BF16)
    lw1 = din("lw1", [20, 2, P], BF16)
    lb = din("lb", [P, 2], FP32)
    leakst = din("leakst", [148, BCH], BF16)

    out = nc.declare_dram_parameter("out", [1, BCH], FP32, True)


    with tile.TileContext(nc) as tc, tc.tile_pool(name="const", bufs=1) as cpool:
        # ---- constants in SBUF
        ident = cpool.tile([P, P], BF16)
        make_identity(nc, ident[:])
        idx_sb = cpool.tile([P, NCH], I32)
        nc.sync.dma_start(idx_sb[:], idxw[:])
        pb_sb = cpool.tile([P, n_layers, 2, JG], FP32)
        nc.sync.dma_start(pb_sb[:], pbias[:])
        b1h_sb = cpool.tile([P, n_layers, 2, JH, BC], BF16)
        nc.sync.dma_start(b1h_sb[:], b1hbc[:])
        fin_hold = [cpool.tile([P, JH, 1, BC], BF16, name=f"fin{i}") for i in range(2)]
        ones_sb = cpool.tile([P, JH, BC], FP32, name="ones_sb")
        nc.vector.memset(ones_sb[:], 1.0)
        # layer activations and input projections live entirely in SBUF:
        # x_sb [P, 4, T, BC] (single buffer: proj l reads it fully before
        # scan l overwrites it); xp_sb [P, dir, gate_tile, TB, U, BC] with
        # bw stored time-reversed so the scan indexes both dirs identically.
        x_sb = cpool.tile([P, JX, T, BC], BF16, name="x_sb")
        xp_sb = cpool.tile([P, 2, JG, TB, U, BC], BF16, name="xp_sb")

        # ---- phase 1: embedding gather -> x_sb (layer-0 input, transposed)
        with (
            tc.tile_pool(name="erow", bufs=3) as epool,
            tc.tile_pool(name="epsum", bufs=4, space="PSUM") as eppool,
        ):
            for tc_i in range(T // P):
                for bi in range(BC):
                    ch = bi * (T // P) + tc_i
                    g = epool.tile([P, E], BF16)
                    nc.gpsimd.indirect_dma_start(
                        out=g[:],
                        out_offset=None,
                        in_=emb[:],
                        in_offset=bass.IndirectOffsetOnAxis(
                            ap=idx_sb[:, ch : ch + 1], axis=0
                        ),
                    )
                    for j in range(JX):
                        pst = eppool.tile([P, P], BF16)
                        nc.tensor.transpose(pst[:], g[:, j * P : (j + 1) * P], ident[:])
                        dst = x_sb[:, j, tc_i * P : (tc_i + 1) * P, bi]
                        if (bi + j) % 2 == 0:
                            nc.vector.tensor_copy(dst, pst[:])
                        else:
                            nc.scalar.copy(dst, pst[:])

        # ---- per-layer: proj (both dirs) then scan (both dirs)
        for l in range(n_layers):
            is_last = l == n_layers - 1

            # -- input projection: xp^T = Wx^T @ x^T (+bias), psum -> xp_sb
            with (
                tc.tile_pool(name="wts", bufs=1) as wpool,
                tc.tile_pool(name="ppsum", bufs=1, space="PSUM") as pppool,
            ):
                wx_sb = wpool.tile([P, 2, JX, JG, P], BF16)
                nc.sync.dma_start(wx_sb[:], wx[l].rearrange("d kt mt p q -> p d kt mt q"))

                NCK = T // 64  # chunks of 512 psum cols (4 tb x 16 u x 8 b)
                NTBC = 64 // U  # tb blocks per chunk (4)
                CPG = min(4, NCK)  # chunks per group: one ldw serves CPG matmuls
                for d in range(2):
                    for mt in range(JG):
                        for cg in range(NCK // CPG):
                            pss = [
                                pppool.tile([P, 512], FP32, tag=f"pp{ci}", name=f"pp{ci}")
                                for ci in range(CPG)
                            ]
                            for kt in range(JX):
                                for ci in range(CPG):
                                    c = cg * CPG + ci
                                    nc.tensor.matmul(
                                        pss[ci][:],
                                        wx_sb[:, d, kt, mt, :],
                                        x_sb[:, kt, c * 64 : (c + 1) * 64, :],
                                        start=(kt == 0),
                                        stop=(kt == JX - 1),
                                    )
                            for ci in range(CPG):
                                c = cg * CPG + ci
                                # psum col order = (tb, u, b) -> xp_sb slice
                                if d == 0:
                                    dst = xp_sb[
                                        :, 0, mt, c * NTBC : (c + 1) * NTBC, :, :
                                    ]
                                else:
                                    # bw: store reversed in time (block and
                                    # within-block order both reversed)
                                    dst = xp_sb[
                                        :, 1, mt,
                                        TB - (c + 1) * NTBC : TB - c * NTBC, :, :,
                                    ][:, ::-1, ::-1, :]
                                if mt < 4:
                                    nc.vector.tensor_scalar_add(
                                        dst, pss[ci][:], pb_sb[:, l, d, mt : mt + 1]
                                    )
                                else:
                                    nc.scalar.activation(
                                        dst,
                                        pss[ci][:],
                                        AF.Identity,
                                        bias=pb_sb[:, l, d, mt : mt + 1],
                                    )

            # -- scan
            with (
                tc.tile_pool(name="state", bufs=1) as stpool,
                tc.tile_pool(name="gates", bufs=4) as gpool,
                tc.tile_pool(name="spsum", bufs=2, space="PSUM") as sppool,
                tc.tile_pool(name="wts2", bufs=1) as wpool2,
            ):
                wh_sb = wpool2.tile([P, 2, KH, JG, P], BF16)
                nc.sync.dma_start(wh_sb[:], wh[l].rearrange("d kt mt p q -> p d kt mt q"))

                # state ring [P, KH, U, BC]: fw writes step u to col u; bw
                # writes step u to col U-1-u so ring cols are ascending in t
                # and the x_next DMA is a plain 3D copy. Carry-in: fw reads
                # col U-1 of the previous iteration at u=0, bw reads col 0.
                # memset once: t=0 reads zeros.
                stg_st = []
                for d in range(2):
                    s = stpool.tile([P, KH, U, BC], BF16, name=f"ring{d}")
                    nc.vector.memset(s[:], 0.0)
                    stg_st.append(s)

                def rcol(d, u):
                    # ring column holding the state produced by step u
                    return u if d == 0 else U - 1 - u

                def scan_block(ib):
                    def emit_xnext(d, half):
                        # ring cols [half*8, half*8+8), ascending in t for
                        # both dirs -> on-chip copy into x_sb
                        lo = half * 8
                        src = stg_st[d][:, :, lo : lo + 8, :]
                        if d == 0:
                            t0 = ib * U + lo
                        else:
                            t0 = T - U + lo - ib * U
                        dst = x_sb[:, 2 * d : 2 * d + 2, t0 : t0 + 8, :]
                        nc.gpsimd.tensor_copy(dst, src)

                    for u in range(U):
                        # r-gates get their own small psum tile and are
                        # computed FIRST so sig_r -> hm starts ~600ns before
                        # the full burst would finish; z-gates are off the
                        # critical path (t1 = z*h has slack).
                        prs, pzhs = [], []
                        for d in range(2):
                            up = rcol(d, u - 1) if u > 0 else rcol(d, U - 1)
                            pr = sppool.tile([P, JH, BC], FP32, tag=f"pr{d}")
                            nc.tensor.matmul(
                                pr[:], ident[:], xp_sb[:, d, 2:4, ib, u, :],
                                start=True, stop=False, skip_group_check=True,
                            )
                            for mi in range(JH):
                                for kt in range(KH):
                                    nc.tensor.matmul(
                                        pr[:, mi, :],
                                        wh_sb[:, d, kt, 2 + mi, :],
                                        stg_st[d][:, kt, up, :],
                                        start=False,
                                        stop=(kt == KH - 1),
                                        skip_group_check=True,
                                    )
                            prs.append(pr)
                        for d in range(2):
                            up = rcol(d, u - 1) if u > 0 else rcol(d, U - 1)
                            pzh = sppool.tile([P, 4, BC], FP32, tag=f"pzh{d}")
                            # cols 0:2 = z pre-activations, cols 2:4 = rec_h+b1h
                            nc.tensor.matmul(
                                pzh[:, 2:4, :], ident[:], b1h_sb[:, l, d, :, :],
                                start=True, stop=False, skip_group_check=True,
                            )
                            for mi in range(JH):
                                for kt in range(KH):
                                    nc.tensor.matmul(
                                        pzh[:, 2 + mi, :],
                                        wh_sb[:, d, kt, 4 + mi, :],
                                        stg_st[d][:, kt, up, :],
                                        start=False,
                                        stop=(kt == KH - 1),
                                        skip_group_check=True,
                                    )
                            nc.tensor.matmul(
                                pzh[:, 0:2, :], ident[:], xp_sb[:, d, 0:2, ib, u, :],
                                start=True, stop=False, skip_group_check=True,
                            )
                            for mi in range(JH):
                                for kt in range(KH):
                                    nc.tensor.matmul(
                                        pzh[:, mi, :],
                                        wh_sb[:, d, kt, mi, :],
                                        stg_st[d][:, kt, up, :],
                                        start=False,
                                        stop=(kt == KH - 1),
                                        skip_group_check=True,
                                    )
                            pzhs.append(pzh)
                        rrs, zzs, zcs, t1s, hms, avs, hhs = [], [], [], [], [], [], []
                        for d in range(2):
                            rr = gpool.tile([P, JH, BC], FP32, tag=f"rr{d}")
                            nc.scalar.activation(rr[:], prs[d][:], AF.Sigmoid)
                            rrs.append(rr)
                        for d in range(2):
                            hm = gpool.tile([P, JH, BC], FP32, tag=f"hm{d}")
                            nc.vector.tensor_tensor(
                                hm[:], pzhs[d][:, 2:4, :], rrs[d][:], OP.mult
                            )
                            hms.append(hm)
                        for d in range(2):
                            zz = gpool.tile([P, JH, BC], FP32, tag=f"zz{d}")
                            nc.scalar.activation(zz[:], pzhs[d][:, 0:2, :], AF.Sigmoid)
                            zzs.append(zz)
                        for d in range(2):
                            # on DVE right behind hm: same-queue back-to-back,
                            # no cross-engine semaphore hop on the chain
                            av = gpool.tile([P, JH, BC], FP32, tag=f"av{d}")
                            nc.vector.tensor_tensor(
                                av[:], hms[d][:], xp_sb[:, d, 4:6, ib, u, :], OP.add
                            )
                            avs.append(av)
                        for d in range(2):
                            zc = gpool.tile([P, JH, BC], FP32, tag=f"zc{d}")
                            nc.vector.scalar_tensor_tensor(
                                zc[:], zzs[d][:], -1.0, ones_sb[:], OP.mult, OP.add
                            )
                            zcs.append(zc)
                        for d in range(2):
                            up = rcol(d, u - 1) if u > 0 else rcol(d, U - 1)
                            t1 = gpool.tile([P, JH, BC], FP32, tag=f"t1{d}")
                            nc.gpsimd.tensor_tensor(
                                t1[:], zzs[d][:], stg_st[d][:, :, up, :], OP.mult
                            )
                            t1s.append(t1)
                        for d in range(2):
                            hh = gpool.tile([P, JH, BC], FP32, tag=f"hh{d}")
                            nc.scalar.activation(hh[:], avs[d][:], AF.Tanh)
                            hhs.append(hh)
                        t2s = []
                        for d in range(2):
                            t2 = gpool.tile([P, JH, BC], FP32, tag=f"t2{d}")
                            nc.vector.tensor_tensor(
                                t2[:], zcs[d][:], hhs[d][:], OP.mult
                            )
                            t2s.append(t2)
                        for d in range(2):
                            nc.vector.tensor_tensor(
                                stg_st[d][:, :, rcol(d, u), :],
                                t1s[d][:],
                                t2s[d][:],
                                OP.add,
                            )
                        if not is_last and u == U // 2 - 1:
                            # fw has filled cols 0:8, bw cols 8:16
                            emit_xnext(0, 0)
                            emit_xnext(1, 1)
                    if not is_last:
                        emit_xnext(0, 1)
                        emit_xnext(1, 0)

                for ib in range(TB):
                    scan_block(ib)

                if is_last:
                    for d in range(2):
                        nc.vector.tensor_copy(
                            fin_hold[d][:, :, 0, :],
                            stg_st[d][:, :, rcol(d, U - 1), :],
                        )

        # ---- head: leaks branch + folded BN/FC/BN/sigmoid
        with (
            tc.tile_pool(name="head", bufs=1) as hpool,
            tc.tile_pool(name="hpsum", bufs=2, space="PSUM") as hppool,
        ):
            lkw0 = hpool.tile([P, 2, P], BF16)
            nc.sync.dma_start(lkw0[:], lw0[:])
            lkw1 = hpool.tile([20, 2, P], BF16)
            nc.sync.dma_start(lkw1[:], lw1[:])
            lkb = hpool.tile([P, 2], FP32)
            nc.sync.dma_start(lkb[:], lb[:])
            lkx0 = hpool.tile([P, BCH], BF16)
            nc.sync.dma_start(lkx0[:], leakst[0:P, :])
            lkx1 = hpool.tile([20, BCH], BF16)
            nc.sync.dma_start(lkx1[:], leakst[P:148, :])

            lks = hpool.tile([P, 2, BCH], BF16)
            for mt in range(2):
                lp = hppool.tile([P, BCH], FP32, tag="lp")
                nc.tensor.matmul(lp[:], lkw0[:, mt, :], lkx0[:], start=True, stop=False)
                nc.tensor.matmul(lp[:], lkw1[:, mt, :], lkx1[:], start=False, stop=True)
                nc.scalar.activation(
                    lks[:, mt, :], lp[:], AF.Relu, bias=lkb[:, mt : mt + 1]
                )

            w1_sb = hpool.tile([P, 10, 2, P], BF16)
            nc.sync.dma_start(w1_sb[:], w1[:].rearrange("kt mt p q -> p kt mt q"))
            b1_sb = hpool.tile([P, 2], FP32)
            nc.sync.dma_start(b1_sb[:], b1p[:])
            wc_sb = hpool.tile([P, 2], BF16)
            nc.sync.dma_start(wc_sb[:], wc[:])
            bc_sb = hpool.tile([1, 1], FP32)
            nc.sync.dma_start(bc_sb[:], bc_b[:])

            sf, sb_ = fin_hold
            rhs_tiles = []
            for half in range(2):  # code (cols 0:4), comment (cols 4:8)
                c0 = half * BCH
                for dstate in (sf, sb_):
                    for j in range(JH):
                        rhs_tiles.append(dstate[:, j, 0, c0 : c0 + BCH])
            rhs_tiles.append(lks[:, 0, :])
            rhs_tiles.append(lks[:, 1, :])

            yt = hpool.tile([P, 2, BCH], BF16)
            for mt in range(2):
                hp = hppool.tile([P, BCH], FP32, tag="hp")
                for kt in range(10):
                    nc.tensor.matmul(
                        hp[:],
                        w1_sb[:, kt, mt, :],
                        rhs_tiles[kt],
                        start=(kt == 0),
                        stop=(kt == 9),
                    )
                nc.scalar.activation(
                    yt[:, mt, :], hp[:], AF.Relu, bias=b1_sb[:, mt : mt + 1]
                )

            op_ = hppool.tile([1, BCH], FP32, tag="op")
            for kt in range(2):
                nc.tensor.matmul(
                    op_[:],
                    wc_sb[:, kt : kt + 1],
                    yt[:, kt, :],
                    start=(kt == 0),
                    stop=(kt == 1),
                )
            res = hpool.tile([1, BCH], FP32)
            nc.scalar.activation(res[:], op_[:], AF.Sigmoid, bias=bc_sb[0:1, 0:1])
            nc.sync.dma_start(out[:], res[:])

    nc.compile()
    return nc


def prep_inputs(inputs, T=512, n_layers=NLAY):
    """Host-side: shard + pre-layout all tensors. Returns in_maps list."""
    ci = np.asarray(inputs["comment_indices"]).astype(np.int32)
    co = np.asarray(inputs["code_indices"]).astype(np.int32)
    emb_bf = np.ascontiguousarray(
        np.asarray(inputs["embed_table"], np.float32)
    ).astype(NP_BF16)
    gwx = np.asarray(inputs["gru_Wx"], np.float32)
    gwh = np.asarray(inputs["gru_Wh"], np.float32)
    gb = np.asarray(inputs["gru_b"], np.float32)

    wx_t = np.ascontiguousarray(
        gwx[:n_layers].reshape(n_layers, 2, JX, P, JG, P).transpose(0, 1, 2, 4, 3, 5)
    ).astype(NP_BF16)
    wh_t = np.ascontiguousarray(
        gwh[:n_layers].reshape(n_layers, 2, KH, P, JG, P).transpose(0, 1, 2, 4, 3, 5)
    ).astype(NP_BF16)

    pb = gb[:n_layers, :, 0, :].copy()  # [nl, 2, 768]
    pb[:, :, : 2 * H] += gb[:n_layers, :, 1, : 2 * H]
    pbias_h = np.ascontiguousarray(
        pb.reshape(n_layers, 2, JG, P).transpose(3, 0, 1, 2)
    ).astype(np.float32)
    b1h = np.ascontiguousarray(
        gb[:n_layers, :, 1, 2 * H :].reshape(n_layers, 2, JH, P).transpose(3, 0, 1, 2)
    ).astype(np.float32)  # [P, nl, 2, JH]
    b1hbc_h = np.ascontiguousarray(
        np.broadcast_to(b1h[..., None], (P, n_layers, 2, JH, BC))
    ).astype(NP_BF16)

    s1 = np.asarray(inputs["bn1_gamma"], np.float32) / np.sqrt(
        np.asarray(inputs["bn1_var"], np.float32) + EPS
    )
    t1 = (
        np.asarray(inputs["bn1_beta"], np.float32)
        - np.asarray(inputs["bn1_mean"], np.float32) * s1
    )
    fc1 = np.asarray(inputs["fc1_W"], np.float32)
    w1p = fc1 * s1[:, None]
    b1v = t1 @ fc1 + np.asarray(inputs["fc1_b"], np.float32)
    s2 = np.asarray(inputs["bn2_gamma"], np.float32) / np.sqrt(
        np.asarray(inputs["bn2_var"], np.float32) + EPS
    )
    t2 = (
        np.asarray(inputs["bn2_beta"], np.float32)
        - np.asarray(inputs["bn2_mean"], np.float32) * s2
    )
    clsw = np.asarray(inputs["cls_W"], np.float32)
    wcp = clsw * s2[:, None]
    bcp = (t2 @ clsw + np.asarray(inputs["cls_b"], np.float32)).reshape(1, 1)

    w1_t = np.ascontiguousarray(w1p.reshape(10, P, 2, P).transpose(0, 2, 1, 3)).astype(
        NP_BF16
    )
    b1p_h = np.ascontiguousarray(b1v.reshape(2, P).T).astype(np.float32)
    wc_h = np.ascontiguousarray(wcp.reshape(2, P).T).astype(NP_BF16)

    lw = np.asarray(inputs["leaks_W"], np.float32)
    lw0_h = np.ascontiguousarray(lw[:P].reshape(P, 2, P)).astype(NP_BF16)
    lw1_h = np.ascontiguousarray(lw[P:].reshape(20, 2, P)).astype(NP_BF16)
    lb_h = np.ascontiguousarray(
        np.asarray(inputs["leaks_b"], np.float32).reshape(2, P).T
    ).astype(np.float32)
    leaks = np.asarray(inputs["leaks_indices"], np.float32)

    shared = dict(
        emb=emb_bf, wx=wx_t, wh=wh_t, pbias=pbias_h, b1hbc=b1hbc_h,
        w1=w1_t, b1p=b1p_h, wc=wc_h, bc=bcp.astype(np.float32),
        lw0=lw0_h, lw1=lw1_h, lb=lb_h,
    )
    in_maps = []
    for c in range(NCORES):
        exs = slice(BCH * c, BCH * c + BCH)
        merged = np.concatenate([co[exs, :T], ci[exs, :T]], 0)  # [8, T]
        idxw_h = np.ascontiguousarray(
            merged.reshape(BC, T // P, P).transpose(2, 0, 1).reshape(P, -1)
        ).astype(np.int32)
        lkt = np.ascontiguousarray(leaks[exs].T).astype(NP_BF16)
        m = dict(shared)
        m["idxw"] = idxw_h
        m["leakst"] = lkt
        in_maps.append(m)
    return in_maps


def kernel(**inputs) -> np.ndarray:
    from concourse.bass_utils import run_bass_kernel_spmd

    nc = build_nc(T=512)
    in_maps = prep_inputs(inputs, T=512)
    res = run_bass_kernel_spmd(nc, in_maps, list(range(NCORES)))
    outs = [np.asarray(res.results[c]["out"]).reshape(-1) for c in range(NCORES)]
    return np.concatenate(outs).astype(np.float32)


if __name__ == "__main__":
    sys.path.insert(0, "/root/problem")
    import reference

    inp = {k: np.asarray(v) for k, v in reference.setup_inputs().items()}
    got = kernel(**inp)
    print("kernel out:", got[:8])



# revision 2
# speedup vs baseline: 1.0342x; 1.0342x over previous
"""Trainium2 Bass kernel for nn_BaselineModel_27298812133937.

Model: two [32,512] token sequences -> shared embedding [50000,512] ->
3 stacked bi-GRU layers (H=256, Keras reset_after) -> last states,
plus a leaks MLP branch, then BN/FC/BN/sigmoid head -> [32].

Sharding: the two sequences share GRU weights, so they merge into a
batch of 64. Each of the 8 cores takes 8 merged examples (4 code + 4
comment of the same original examples), runs the full network for its
shard with no cross-core communication, and computes the head for its
4 original examples. Host concatenates the 8x[4] outputs.

Design (measured 3.81 ms vs the 4.16 ms previous best; rel err 1.1e-3).
The scan is gate-chain LATENCY bound, not PE-load bound: matmul+
ldweights pairs issue at ~26ns (ldweights pipelined, 2 rows/cycle
regardless of dtype -- fp8 weights buy nothing), while ACT ops cost
~290-320ns and DVE ops ~190ns each, non-pipelined, under a periodic
~0.5 utilization duty-cycle throttle (88us full speed / 1.27ms at 50%,
environmental; 4-core probe shows it is not chip-power driven).
Scan structure (one merged chain for both directions; ops are
[P, 2dir, tile, batch] so each hop covers both dirs):
- serial loop: v'(t-1) -> v-r matmuls -> sig_r -> hm -> av -> tanh ->
  v'(t), ~2.2us/step; everything else hides in its shadow.
- u/v-split feeding for r,z gates: next step's matmuls read t1=z*h and
  v'=(z-1)*hh as separate rhs (PSUM adds them; the v-side uses a
  negated weight copy so one fused scalar_tensor_tensor produces v'
  straight off tanh, no (1-z) op, no h-add on the loop).
- h gate reads h(t-1) directly (hadd on DVE right after v' lands just
  in time), halving its matmul count.
- t1 on DVE pinned AFTER av via an ordering-only dep (the static
  scheduler otherwise hoists it before av, putting sig_z's latency on
  the main path); nothing latency-critical on gpsimd (its semaphore
  reception is ~400-900ns).
- psum per gate group in separate tiles (tile-granular deps: sig_r
  fires after just the 8 v-r matmuls, not the whole burst).
- projection emits chunk groups d0-ascending / d1-descending
  interleaved so each layer's scan starts ~20us after the boundary;
  per-layer weight DMAs stay inside their phase (prefetching them
  during the scan shifted SBUF layout and inflated every op ~20%).
"""

import os
import sys

import numpy as np

for _p in ("/opt/trn_rl_repo",):
    if os.path.isdir(_p) and _p not in sys.path:
        sys.path.insert(0, _p)

import concourse.bass as bass
import concourse.tile as tile
from concourse import bacc, mybir
from concourse.masks import make_identity

import ml_dtypes

FP32 = mybir.dt.float32
BF16 = mybir.dt.bfloat16
I32 = mybir.dt.int32
AF = mybir.ActivationFunctionType
OP = mybir.AluOpType
NP_BF16 = ml_dtypes.bfloat16

V, E, H, NLAY = 50000, 512, 256, 3
EPS = 1e-3
P = 128
JX = E // P        # 4  x-feature tiles
JG = 3 * H // P    # 6  gate tiles
JH = H // P        # 2  hidden tiles
KH = H // P        # 2  Wh contraction tiles
BC = 8             # merged examples per core
BCH = 4            # head (original) examples per core
NCORES = 8
U = 16             # xp time-block


def build_nc(T=512, n_layers=NLAY, debug=False):
    assert T % P == 0 and T % U == 0
    TB = T // U
    NCH = BC * (T // P)

    nc = bacc.Bacc("TRN2", target_bir_lowering=False, debug=debug)

    def din(name, shape, dt):
        return nc.declare_dram_parameter(name, list(shape), dt, False)

    emb = din("emb", [V, E], BF16)
    idxw = din("idxw", [P, NCH], I32)
    wx = din("wx", [n_layers, 2, JX, JG, P, P], BF16)
    wh = din("wh", [n_layers, 2, KH, JG, P, P], BF16)
    whn = din("whn", [n_layers, 2, KH, JG, P, P], BF16)  # -Wh for the v-side
    pbias = din("pbias", [P, n_layers, 2, JG], FP32)
    b1hbc = din("b1hbc", [P, n_layers, 2, JH, BC], BF16)
    w1 = din("w1", [10, 2, P, P], BF16)
    b1p = din("b1p", [P, 2], FP32)
    wc = din("wc", [P, 2], BF16)
    bc_b = din("bc", [1, 1], FP32)
    lw0 = din("lw0", [P, 2, P], BF16)
    lw1 = din("lw1", [20, 2, P], BF16)
    lb = din("lb", [P, 2], FP32)
    leakst = din("leakst", [148, BCH], BF16)

    out = nc.declare_dram_parameter("out", [1, BCH], FP32, True)

    with tile.TileContext(nc) as tc, tc.tile_pool(name="const", bufs=1) as cpool:
        # ---- constants in SBUF
        ident = cpool.tile([P, P], BF16)
        make_identity(nc, ident[:])
        idx_sb = cpool.tile([P, NCH], I32)
        nc.sync.dma_start(idx_sb[:], idxw[:])
        pb_sb = cpool.tile([P, n_layers, 2, JG], FP32)
        nc.sync.dma_start(pb_sb[:], pbias[:])
        b1h_sb = cpool.tile([P, n_layers, 2, JH, BC], BF16)
        nc.sync.dma_start(b1h_sb[:], b1hbc[:])
        fin_hold = cpool.tile([P, 2, KH, BC], BF16, name="fin")
        # layer activations and input projections live entirely in SBUF:
        # x_sb [P, 4, T, BC]; xp_sb [P, dir, gate_tile, TB, U, BC] with bw
        # stored time-reversed so the scan indexes both dirs identically.
        x_sb = cpool.tile([P, JX, T, BC], BF16, name="x_sb")
        xp_sb = cpool.tile([P, 2, JG, TB, U, BC], BF16, name="xp_sb")

        # ---- phase 1: embedding gather -> x_sb (layer-0 input, transposed)
        with (
            tc.tile_pool(name="erow", bufs=3) as epool,
            tc.tile_pool(name="epsum", bufs=4, space="PSUM") as eppool,
        ):
            for tc_i in range(T // P):
                for bi in range(BC):
                    ch = bi * (T // P) + tc_i
                    g = epool.tile([P, E], BF16)
                    nc.gpsimd.indirect_dma_start(
                        out=g[:],
                        out_offset=None,
                        in_=emb[:],
                        in_offset=bass.IndirectOffsetOnAxis(
                            ap=idx_sb[:, ch : ch + 1], axis=0
                        ),
                    )
                    for j in range(JX):
                        pst = eppool.tile([P, P], BF16)
                        nc.tensor.transpose(pst[:], g[:, j * P : (j + 1) * P], ident[:])
                        dst = x_sb[:, j, tc_i * P : (tc_i + 1) * P, bi]
                        if (bi + j) % 2 == 0:
                            nc.vector.tensor_copy(dst, pst[:])
                        else:
                            nc.scalar.copy(dst, pst[:])

        # ---- per-layer: proj (both dirs) then scan (both dirs)
        for l in range(n_layers):
            is_last = l == n_layers - 1

            # -- input projection: xp^T = Wx^T @ x^T (+bias), psum -> xp_sb
            with (
                tc.tile_pool(name="wts", bufs=1) as wpool,
                tc.tile_pool(name="ppsum", bufs=1, space="PSUM") as pppool,
            ):
                wx_sb = wpool.tile([P, 2, JX, JG, P], BF16)
                nc.sync.dma_start(wx_sb[:], wx[l].rearrange("d kt mt p q -> p d kt mt q"))

                NCK = T // 64  # chunks of 512 psum cols (4 tb x 16 u x 8 b)
                NTBC = 64 // U  # tb blocks per chunk (4)
                CPG = min(4, NCK)  # chunks per group: one ldw serves CPG matmuls
                NG = NCK // CPG
                # order: d0 ascending, d1 descending (bw xp is stored
                # time-reversed: its last chunk group holds the scan's first
                # blocks), interleaved so both dirs' first-needed xp lands
                # first
                dcg_order = []
                for gi in range(NG):
                    dcg_order.append((0, gi))
                    dcg_order.append((1, NG - 1 - gi))
                for d, cg in dcg_order:
                    for mt in range(JG):
                        if True:
                            pss = [
                                pppool.tile([P, 512], FP32, tag=f"pp{ci}", name=f"pp{ci}")
                                for ci in range(CPG)
                            ]
                            for kt in range(JX):
                                for ci in range(CPG):
                                    c = cg * CPG + ci
                                    nc.tensor.matmul(
                                        pss[ci][:],
                                        wx_sb[:, d, kt, mt, :],
                                        x_sb[:, kt, c * 64 : (c + 1) * 64, :],
                                        start=(kt == 0),
                                        stop=(kt == JX - 1),
                                    )
                            for ci in range(CPG):
                                c = cg * CPG + ci
                                # psum col order = (tb, u, b) -> xp_sb slice
                                if d == 0:
                                    dst = xp_sb[
                                        :, 0, mt, c * NTBC : (c + 1) * NTBC, :, :
                                    ]
                                else:
                                    # bw: store reversed in time (block and
                                    # within-block order both reversed)
                                    dst = xp_sb[
                                        :, 1, mt,
                                        TB - (c + 1) * NTBC : TB - c * NTBC, :, :,
                                    ][:, ::-1, ::-1, :]
                                if mt < 4:
                                    nc.vector.tensor_scalar_add(
                                        dst, pss[ci][:], pb_sb[:, l, d, mt : mt + 1]
                                    )
                                else:
                                    nc.scalar.activation(
                                        dst,
                                        pss[ci][:],
                                        AF.Identity,
                                        bias=pb_sb[:, l, d, mt : mt + 1],
                                    )

            # -- scan (merged dirs, u/v-split feeding)
            with (
                tc.tile_pool(name="state", bufs=1) as stpool,
                tc.tile_pool(name="gates", bufs=3) as gpool,
                tc.tile_pool(name="uv", bufs=2) as uvpool,
                tc.tile_pool(name="spsum", bufs=1, space="PSUM") as sppool,
                tc.tile_pool(name="wts2", bufs=1) as wpool2,
            ):
                wh_sb = wpool2.tile([P, 2, KH, JG, P], BF16)
                nc.sync.dma_start(wh_sb[:], wh[l].rearrange("d kt mt p q -> p d kt mt q"))
                whn_sb = wpool2.tile([P, 2, KH, JG, P], BF16)
                nc.sync.dma_start(whn_sb[:], whn[l].rearrange("d kt mt p q -> p d kt mt q"))
                # unified ring: both dirs write col u (bw time-reversed).
                hring = stpool.tile([P, 2, KH, U, BC], BF16, name="hring")
                nc.vector.memset(hring[:], 0.0)

                def new_uv(t):
                    u_t = uvpool.tile([P, 2, KH, BC], BF16, tag=f"u{t % 2}")
                    v_t = uvpool.tile([P, 2, KH, BC], BF16, tag=f"v{t % 2}")
                    return u_t, v_t

                def new_psum(t):
                    # separate tiles per gate group so readers only wait on
                    # their own group's matmuls (tile-granular deps)
                    pr = sppool.tile([P, 2, JH, BC], FP32, tag=f"pr{t % 2}")
                    pz = sppool.tile([P, 2, JH, BC], FP32, tag=f"pz{t % 2}")
                    ph = sppool.tile([P, 2, JH, BC], FP32, tag=f"ph{t % 2}")
                    return pr, pz, ph

                def emit_idents(ps, ib, u):
                    # one ident LDWEIGHTS serves all three injections
                    # (consecutive same-lhsT matmuls)
                    pr, pz, ph = ps
                    nc.tensor.matmul(
                        pr[:], ident[:], xp_sb[:, :, 2:4, ib, u, :],
                        start=True, stop=False, skip_group_check=True,
                    )
                    nc.tensor.matmul(
                        pz[:], ident[:], xp_sb[:, :, 0:2, ib, u, :],
                        start=True, stop=False, skip_group_check=True,
                    )
                    nc.tensor.matmul(
                        ph[:], ident[:], b1h_sb[:, l, :, :, :],
                        start=True, stop=False, skip_group_check=True,
                    )

                def emit_g_mms(rhs, dst, goff, stop, W):
                    # one gate group (8 matmuls)
                    for mi in range(JH):
                        for d in range(2):
                            for kt in range(KH):
                                nc.tensor.matmul(
                                    dst[:, d, mi, :],
                                    W[:, d, kt, goff + mi, :],
                                    rhs[:, d, kt, :],
                                    start=False, stop=(stop and kt == KH - 1),
                                    skip_group_check=True,
                                )

                def emit_hh_mms(hcol, ph):
                    # h gate reads h(t-1) directly (hadd lands just in time)
                    for mi in range(JH):
                        for d in range(2):
                            for kt in range(KH):
                                nc.tensor.matmul(
                                    ph[:, d, mi, :],
                                    wh_sb[:, d, kt, 4 + mi, :],
                                    hring[:, d, kt, hcol, :],
                                    start=False, stop=(kt == KH - 1),
                                    skip_group_check=True,
                                )

                def emit_x(ib, half):
                    # ring cols [half*8, half*8+8); fw ascending in t, bw
                    # descending (reversed AP on the copy)
                    lo = half * 8
                    t0 = ib * U + lo
                    nc.gpsimd.tensor_copy(
                        x_sb[:, 0:2, t0 : t0 + 8, :],
                        hring[:, 0, :, lo : lo + 8, :],
                    )
                    hi = T - ib * U - lo
                    nc.gpsimd.tensor_copy(
                        x_sb[:, 2:4, hi - 8 : hi, :],
                        hring[:, 1, :, lo : lo + 8, :][:, :, ::-1, :],
                    )

                # seed t=0: zero u/v (h_-1 = 0), psum + idents + u-r MMs
                u_prev, v_prev = new_uv(0)
                nc.vector.memset(u_prev[:], 0.0)
                nc.gpsimd.memset(v_prev[:], 0.0)
                ps = new_psum(0)
                emit_idents(ps, 0, 0)
                emit_g_mms(u_prev, ps[0], 2, stop=False, W=wh_sb)
                emit_g_mms(u_prev, ps[1], 0, stop=False, W=wh_sb)

                for t in range(T):
                    ib, u = divmod(t, U)
                    up = u - 1 if u > 0 else U - 1  # ring col of h(t-1)
                    pr_t, pz_t, ph_t = ps
                    # burst order: v-r (sig_r path), hh (hm path), v-z
                    emit_g_mms(v_prev, pr_t, 2, stop=True, W=whn_sb)
                    emit_hh_mms(up, ph_t)
                    emit_g_mms(v_prev, pz_t, 0, stop=True, W=whn_sb)
                    # gate chain (merged dirs)
                    srz = gpool.tile([P, 2, 4, BC], FP32, tag="srz")
                    nc.scalar.activation(srz[:, :, 2:4, :], pr_t[:], AF.Sigmoid)
                    nc.scalar.activation(srz[:, :, 0:2, :], pz_t[:], AF.Sigmoid)
                    hm = gpool.tile([P, 2, JH, BC], FP32, tag="hm")
                    nc.vector.tensor_tensor(
                        hm[:], ph_t[:], srz[:, :, 2:4, :], OP.mult
                    )
                    av = gpool.tile([P, 2, JH, BC], FP32, tag="av")
                    av_i = nc.vector.tensor_tensor(
                        av[:], hm[:], xp_sb[:, :, 4:6, ib, u, :], OP.add
                    )
                    hh = gpool.tile([P, 2, JH, BC], FP32, tag="hh")
                    nc.scalar.activation(hh[:], av[:], AF.Tanh)
                    u_next, v_next = new_uv(t + 1)
                    # t1 = z*h(t-1) on DVE, pinned AFTER av (ordering-only
                    # edge): the static scheduler otherwise hoists it before
                    # av, putting sig_z's latency on the main path
                    t1_i = nc.vector.tensor_tensor(
                        u_next[:], srz[:, :, 0:2, :], hring[:, :, :, up, :], OP.mult
                    )
                    tile.add_dep_helper(t1_i.ins, av_i.ins, sync=False,
                                        reason="pin t1 after av")
                    # v' = (z-1)*hh in ONE DVE op; the v-matmuls use -Wh so
                    # PSUM accumulation implements Wh*(1-z)*hh
                    nc.vector.scalar_tensor_tensor(
                        v_next[:], srz[:, :, 0:2, :], 1.0, hh[:],
                        OP.subtract, OP.mult,
                    )
                    # h = t1 - v' = z*h + (1-z)*hh (DVE, right after v'; feeds
                    # next step's hz/hh matmuls)
                    nc.vector.tensor_tensor(
                        hring[:, :, :, u, :], u_next[:], v_next[:], OP.subtract
                    )
                    # next step's psum init + u-r MMs (issued into this step's
                    # chain shadow)
                    if t < T - 1:
                        ib_n, u_n = divmod(t + 1, U)
                        ps_n = new_psum(t + 1)
                        emit_idents(ps_n, ib_n, u_n)
                        emit_g_mms(u_next, ps_n[0], 2, stop=False, W=wh_sb)
                        emit_g_mms(u_next, ps_n[1], 0, stop=False, W=wh_sb)
                        ps = ps_n
                    u_prev, v_prev = u_next, v_next
                    if not is_last and u in (U // 2 - 1, U - 1):
                        emit_x(ib, 0 if u == U // 2 - 1 else 1)

                if is_last:
                    nc.vector.tensor_copy(fin_hold[:], hring[:, :, :, U - 1, :])

        # ---- head: leaks branch + folded BN/FC/BN/sigmoid
        with (
            tc.tile_pool(name="head", bufs=1) as hpool,
            tc.tile_pool(name="hpsum", bufs=2, space="PSUM") as hppool,
        ):
            lkw0 = hpool.tile([P, 2, P], BF16)
            nc.sync.dma_start(lkw0[:], lw0[:])
            lkw1 = hpool.tile([20, 2, P], BF16)
            nc.sync.dma_start(lkw1[:], lw1[:])
            lkb = hpool.tile([P, 2], FP32)
            nc.sync.dma_start(lkb[:], lb[:])
            lkx0 = hpool.tile([P, BCH], BF16)
            nc.sync.dma_start(lkx0[:], leakst[0:P, :])
            lkx1 = hpool.tile([20, BCH], BF16)
            nc.sync.dma_start(lkx1[:], leakst[P:148, :])

            lks = hpool.tile([P, 2, BCH], BF16)
            for mt in range(2):
                lp = hppool.tile([P, BCH], FP32, tag="lp")
                nc.tensor.matmul(lp[:], lkw0[:, mt, :], lkx0[:], start=True, stop=False)
                nc.tensor.matmul(lp[:], lkw1[:, mt, :], lkx1[:], start=False, stop=True)
                nc.scalar.activation(
                    lks[:, mt, :], lp[:], AF.Relu, bias=lkb[:, mt : mt + 1]
                )

            w1_sb = hpool.tile([P, 10, 2, P], BF16)
            nc.sync.dma_start(w1_sb[:], w1[:].rearrange("kt mt p q -> p kt mt q"))
            b1_sb = hpool.tile([P, 2], FP32)
            nc.sync.dma_start(b1_sb[:], b1p[:])
            wc_sb = hpool.tile([P, 2], BF16)
            nc.sync.dma_start(wc_sb[:], wc[:])
            bc_sb = hpool.tile([1, 1], FP32)
            nc.sync.dma_start(bc_sb[:], bc_b[:])

            rhs_tiles = []
            for half in range(2):  # code (cols 0:4), comment (cols 4:8)
                c0 = half * BCH
                for d in range(2):
                    for j in range(JH):
                        rhs_tiles.append(fin_hold[:, d, j, c0 : c0 + BCH])
            rhs_tiles.append(lks[:, 0, :])
            rhs_tiles.append(lks[:, 1, :])

            yt = hpool.tile([P, 2, BCH], BF16)
            for mt in range(2):
                hp = hppool.tile([P, BCH], FP32, tag="hp")
                for kt in range(10):
                    nc.tensor.matmul(
                        hp[:],
                        w1_sb[:, kt, mt, :],
                        rhs_tiles[kt],
                        start=(kt == 0),
                        stop=(kt == 9),
                    )
                nc.scalar.activation(
                    yt[:, mt, :], hp[:], AF.Relu, bias=b1_sb[:, mt : mt + 1]
                )

            op_ = hppool.tile([1, BCH], FP32, tag="op")
            for kt in range(2):
                nc.tensor.matmul(
                    op_[:],
                    wc_sb[:, kt : kt + 1],
                    yt[:, kt, :],
                    start=(kt == 0),
                    stop=(kt == 1),
                )
            res = hpool.tile([1, BCH], FP32)
            nc.scalar.activation(res[:], op_[:], AF.Sigmoid, bias=bc_sb[0:1, 0:1])
            nc.sync.dma_start(out[:], res[:])

    nc.compile()
    return nc


def prep_inputs(inputs, T=512, n_layers=NLAY):
    """Host-side: shard + pre-layout all tensors. Returns in_maps list."""
    ci = np.asarray(inputs["comment_indices"]).astype(np.int32)
    co = np.asarray(inputs["code_indices"]).astype(np.int32)
    emb_bf = np.ascontiguousarray(
        np.asarray(inputs["embed_table"], np.float32)
    ).astype(NP_BF16)
    gwx = np.asarray(inputs["gru_Wx"], np.float32)
    gwh = np.asarray(inputs["gru_Wh"], np.float32)
    gb = np.asarray(inputs["gru_b"], np.float32)

    wx_t = np.ascontiguousarray(
        gwx[:n_layers].reshape(n_layers, 2, JX, P, JG, P).transpose(0, 1, 2, 4, 3, 5)
    ).astype(NP_BF16)
    wh_t = np.ascontiguousarray(
        gwh[:n_layers].reshape(n_layers, 2, KH, P, JG, P).transpose(0, 1, 2, 4, 3, 5)
    ).astype(NP_BF16)

    pb = gb[:n_layers, :, 0, :].copy()  # [nl, 2, 768]
    pb[:, :, : 2 * H] += gb[:n_layers, :, 1, : 2 * H]
    pbias_h = np.ascontiguousarray(
        pb.reshape(n_layers, 2, JG, P).transpose(3, 0, 1, 2)
    ).astype(np.float32)
    b1h = np.ascontiguousarray(
        gb[:n_layers, :, 1, 2 * H :].reshape(n_layers, 2, JH, P).transpose(3, 0, 1, 2)
    ).astype(np.float32)  # [P, nl, 2, JH]
    b1hbc_h = np.ascontiguousarray(
        np.broadcast_to(b1h[..., None], (P, n_layers, 2, JH, BC))
    ).astype(NP_BF16)

    s1 = np.asarray(inputs["bn1_gamma"], np.float32) / np.sqrt(
        np.asarray(inputs["bn1_var"], np.float32) + EPS
    )
    t1 = (
        np.asarray(inputs["bn1_beta"], np.float32)
        - np.asarray(inputs["bn1_mean"], np.float32) * s1
    )
    fc1 = np.asarray(inputs["fc1_W"], np.float32)
    w1p = fc1 * s1[:, None]
    b1v = t1 @ fc1 + np.asarray(inputs["fc1_b"], np.float32)
    s2 = np.asarray(inputs["bn2_gamma"], np.float32) / np.sqrt(
        np.asarray(inputs["bn2_var"], np.float32) + EPS
    )
    t2 = (
        np.asarray(inputs["bn2_beta"], np.float32)
        - np.asarray(inputs["bn2_mean"], np.float32) * s2
    )
    clsw = np.asarray(inputs["cls_W"], np.float32)
    wcp = clsw * s2[:, None]
    bcp = (t2 @ clsw + np.asarray(inputs["cls_b"], np.float32)).reshape(1, 1)

    w1_t = np.ascontiguousarray(w1p.reshape(10, P, 2, P).transpose(0, 2, 1, 3)).astype(
        NP_BF16
    )
    b1p_h = np.ascontiguousarray(b1v.reshape(2, P).T).astype(np.float32)
    wc_h = np.ascontiguousarray(wcp.reshape(2, P).T).astype(NP_BF16)

    lw = np.asarray(inputs["leaks_W"], np.float32)
    lw0_h = np.ascontiguousarray(lw[:P].reshape(P, 2, P)).astype(NP_BF16)
    lw1_h = np.ascontiguousarray(lw[P:].reshape(20, 2, P)).astype(NP_BF16)
    lb_h = np.ascontiguousarray(
        np.asarray(inputs["leaks_b"], np.float32).reshape(2, P).T
    ).astype(np.float32)
    leaks = np.asarray(inputs["leaks_indices"], np.float32)

    shared = dict(
        emb=emb_bf, wx=wx_t, wh=wh_t, whn=(-wh_t).astype(NP_BF16),
        pbias=pbias_h, b1hbc=b1hbc_h,
        w1=w1_t, b1p=b1p_h, wc=wc_h, bc=bcp.astype(np.float32),
        lw0=lw0_h, lw1=lw1_h, lb=lb_h,
    )
    in_maps = []
    for c in range(NCORES):
        exs = slice(BCH * c, BCH * c + BCH)
        merged = np.concatenate([co[exs, :T], ci[exs, :T]], 0)  # [8, T]
        idxw_h = np.ascontiguousarray(
            merged.reshape(BC, T // P, P).transpose(2, 0, 1).reshape(P, -1)
        ).astype(np.int32)
        lkt = np.ascontiguousarray(leaks[exs].T).astype(NP_BF16)
        m = dict(shared)
        m["idxw"] = idxw_h
        m["leakst"] = lkt
        in_maps.append(m)
    return in_maps


def kernel(**inputs) -> np.ndarray:
    from concourse.bass_utils import run_bass_kernel_spmd

    nc = build_nc(T=512)
    in_maps = prep_inputs(inputs, T=512)
    res = run_bass_kernel_spmd(nc, in_maps, list(range(NCORES)))
    outs = [np.asarray(res.results[c]["out"]).reshape(-1) for c in range(NCORES)]
    return np.concatenate(outs).astype(np.float32)


if __name__ == "__main__":
    sys.path.insert(0, "/root/problem")
    import reference

    inp = {k: np.asarray(v) for k, v in reference.setup_inputs().items()}
    got = kernel(**inp)
    print("kernel out:", got[:8])
